# revision 83
# baseline (speedup 1.0000x reference)
"""GATv2 graph layer Bass kernel for TRN2 (SPMD across 8 NeuronCores, no
collectives).

Design (v2): edges sorted by destination node and sharded across cores by dst
range. Each core builds ONE fp16 gather table in DRAM (xsrc = node_emb @
W_src, split in two halves so int16 gather indices fit), with rows remapped so
table writes use 1KB descriptors. Destination-side features are NOT gathered
from DRAM: x_dst for the core's own nodes lives in SBUF ([P, nw, HID] f16),
and per edge chunk (128 edges, one 128-node dst window) the dst contribution
is gathered by the TensorEngine via a transposed one-hot:

  oh[e, d]  = (dstr[e] == d)                       (DVE tensor_scalar)
  ohT[d, e] = transpose(oh)                        (PE transpose or XBAR DMA)
  psum_combT[hid, e] = xdst_win^T @ ohT            (lhsT=xdst_win)
                     + emb8^T @ ohetT              (host one-hot over 8 types)
                     + xs_chunk^T                  (lhsT=xs, rhs=identity)
  combT16 = Prelu(psum_combT, alpha=0.2)           (Act: fused lrelu + copy)
  ex_ps[e, h] = combT16^T @ att_blk                (PE)
  ex16 = Exp(ex_ps)                                (Act, 8-chunk groups)
  wgt = xs * bcast(ex16)                           (DVE/Pool)
  win_ps[dst, 4+HID] += oh^T @ [ex16 | wgt]        (PE scatter)

Window flush: agg = sum(ex*xs)/sum(ex), @W_out (gamma-scaled) + beta_eff,
residual, LayerNorm via bn_stats/bn_aggr, DMA out. No max-subtraction in the
softmax: logits are bounded so exp stays finite in f32 (matches reference to
~1e-4).
"""
import numpy as np
from contextlib import ExitStack
from dataclasses import dataclass

import concourse.bass as bass
import concourse.tile as tile
from concourse import bacc, mybir
from concourse.masks import make_identity

P = 128
HID = 128
H = 4
HD = 32
NET = 8
EPS_LN = 1e-5
MAXCALL = 4096  # max idxs per dma_gather call
DEAD = -5.0     # dst_rel for padding slots (matches no one-hot column)


@dataclass
class Geo:
    N: int
    n_cores: int
    slab_w: int = 3     # windows per slab
    dma_q: int = 0      # unused (PE transpose path)
    wmul_pool_mod: int = 2  # every k-th chunk's weight-mul runs on DVE

    @property
    def npc(self):
        return self.N // self.n_cores

    @property
    def nw(self):
        return (self.npc + P - 1) // P

    @property
    def nslab(self):
        return (self.nw + self.slab_w - 1) // self.slab_w

    @property
    def n_pad(self):   # padded node count (512 blocks)
        return ((self.N + 511) // 512) * 512

    @property
    def split(self):   # lo/hi table split on a 512 block boundary
        return (self.n_pad // 1024) * 512


def wrap_idx(idx, cols):
    n = idx.shape[0]
    assert n % 16 == 0
    w = np.zeros((P, cols), dtype=np.int16)
    if n:
        t16 = idx.reshape(n // 16, 16).T
        for g in range(8):
            w[g * 16:(g + 1) * 16, :n // 16] = t16
    return w


def remap_row(r):
    """Table row remap so device-side table writes are 1KB/partition:
    original row i*512 + s*128 + p is stored at i*512 + p*4 + s."""
    i, rem = r // 512, r % 512
    return i * 512 + (rem % 128) * 4 + rem // 128


def host_prep(g: Geo, node_embeddings, edge_index, edge_type, task_embedding,
              W_src, b_src, W_dst, b_dst, edge_emb, att,
              W_out, b_out, norm_w, norm_b, W_film, b_film):
    """Returns (sched, in_maps). Pure index work + tiny constant folding;
    all O(N*HID) / O(E*HID) float math runs on device."""
    src = np.asarray(edge_index[0], dtype=np.int64)
    dst = np.asarray(edge_index[1], dtype=np.int64)
    et = np.asarray(edge_type, dtype=np.int64)
    npc, split = g.npc, g.split

    order = np.argsort(dst, kind="stable")
    src, dst, et = src[order], dst[order], et[order]
    core_of = dst // npc

    buckets = {}
    for c in range(g.n_cores):
        m = core_of == c
        cs, cd, ce = src[m], dst[m] - c * npc, et[m]
        for w in range(g.nw):
            wm = (cd // P) == w
            ws_, wd, we = cs[wm], cd[wm] - w * P, ce[wm]
            lo = ws_ < split
            buckets[(c, w, 0)] = (remap_row(ws_[lo]), wd[lo], we[lo])
            buckets[(c, w, 1)] = (remap_row(ws_[~lo] - split), wd[~lo], we[~lo])

    caps = np.zeros((g.nw, 2), dtype=np.int64)
    for w in range(g.nw):
        for h in range(2):
            mx = max(len(buckets[(c, w, h)][0]) for c in range(g.n_cores))
            caps[w, h] = (mx + P - 1) // P

    # ---- schedule ---------------------------------------------------------
    sched_slabs = []
    total_chunks = 0
    for s in range(g.nslab):
        ws = list(range(s * g.slab_w, min((s + 1) * g.slab_w, g.nw)))
        chunks = []            # (win_local, half, slot)
        calls = {0: [], 1: []}
        slot = 0
        for h in (0, 1):
            run = 0
            run_start = slot
            for w in ws:
                for _ in range(caps[w, h]):
                    chunks.append((w - ws[0], h, slot))
                    slot += 1
                    run += P
                    if run == MAXCALL:
                        calls[h].append((run_start, run))
                        run, run_start = 0, slot
            if run:
                calls[h].append((run_start, run))
        sched_slabs.append(dict(windows=ws, chunks=chunks, calls=calls,
                                chunk0=total_chunks))
        total_chunks += len(chunks)

    lo_cols = max(16, sum(n for sl in sched_slabs
                          for (_, n) in sl["calls"][0]) // 16)
    hi_cols = max(16, sum(n for sl in sched_slabs
                          for (_, n) in sl["calls"][1]) // 16)

    # ---- shared constants -------------------------------------------------
    nodeT = np.zeros((HID, g.n_pad), dtype=np.float16)
    nodeT[:, :g.N] = np.asarray(node_embeddings, np.float32).T.astype(np.float16)
    emb_eff = (np.asarray(edge_emb, np.float64)
               + np.asarray(b_src, np.float64)[None, :]
               + np.asarray(b_dst, np.float64)[None, :]).astype(np.float16)
    att_blk = np.zeros((HID, H), dtype=np.float16)
    for h in range(H):
        att_blk[h * HD:(h + 1) * HD, h] = np.asarray(att, np.float32)[h]

    consts = dict(
        nodeT=nodeT,
        W_src=np.asarray(W_src, np.float32).astype(np.float16),
        W_dst=np.asarray(W_dst, np.float32).astype(np.float16),
        W_out=np.asarray(W_out, np.float32).astype(np.float16),
        W_film=np.asarray(W_film, np.float32).astype(np.float16),
        b_film=np.asarray(b_film, np.float32).reshape(1, 2 * HID),
        b_out=np.asarray(b_out, np.float32).reshape(1, HID),
        task=np.asarray(task_embedding, np.float32).reshape(HID, 1)
            .astype(np.float16),
        emb8=emb_eff,                       # [8, HID] f16
        att_blk=att_blk,
    )
    skip_norm = bool(np.all(np.asarray(norm_w) == 1.0)
                     and np.all(np.asarray(norm_b) == 0.0))
    if not skip_norm:
        consts["normw"] = np.asarray(norm_w, np.float32).reshape(1, HID)
        consts["normb"] = np.asarray(norm_b, np.float32).reshape(1, HID)

    node_f16 = np.asarray(node_embeddings, np.float32).astype(np.float16)

    # ---- per-core arrays --------------------------------------------------
    in_maps = []
    for c in range(g.n_cores):
        lo_l, hi_l = [], []
        dstr = np.full((P, total_chunks), DEAD, dtype=np.float32)
        oet = np.zeros((NET, total_chunks * P), dtype=np.float16)
        ci = 0
        for sl in sched_slabs:
            ws0 = sl["windows"][0]
            per_half = {0: [], 1: []}
            nth = {}
            for (wl, h, slot) in sl["chunks"]:
                w = ws0 + wl
                es, ed, ee = buckets[(c, w, h)]
                k = nth.get((wl, h), 0)
                nth[(wl, h)] = k + 1
                sl_src = np.zeros(P, dtype=np.int64)
                n = min(P, max(0, len(es) - k * P))
                if n > 0:
                    sl_src[:n] = es[k * P:k * P + n]
                    dstr[:n, ci] = ed[k * P:k * P + n]
                    oet[ee[k * P:k * P + n], ci * P + np.arange(n)] = 1.0
                per_half[h].append(sl_src)
                ci += 1
            lo_l.extend(per_half[0])
            hi_l.extend(per_half[1])
        lo_i = (np.concatenate(lo_l) if lo_l else np.zeros(0, np.int64))
        hi_i = (np.concatenate(hi_l) if hi_l else np.zeros(0, np.int64))
        assert lo_i.max(initial=0) < g.split <= 32767
        assert hi_i.max(initial=0) < g.n_pad - g.split <= 32768

        own = node_f16[c * npc:(c + 1) * npc]          # [npc, HID] f16
        ownT = np.zeros((HID, g.nw * P), dtype=np.float16)
        ownT[:, :npc] = own.T

        m = dict(consts)
        m["node_own16"] = np.ascontiguousarray(own)
        m["node_ownT"] = ownT
        m["lo_idx"] = wrap_idx(lo_i.astype(np.int16), lo_cols)
        m["hi_idx"] = wrap_idx(hi_i.astype(np.int16), hi_cols)
        m["dstr"] = dstr
        m["ohetT"] = oet
        in_maps.append(m)

    sched = dict(slabs=sched_slabs, caps=caps, total_chunks=total_chunks,
                 lo_cols=lo_cols, hi_cols=hi_cols, skip_norm=skip_norm)
    return sched, in_maps


def build_program(g: Geo, sched, debug=False):
    nc = bacc.Bacc("TRN2", target_bir_lowering=False, debug=False,
                   num_devices=g.n_cores, num_swdge_queues=4)
    f16, f32 = mybir.dt.float16, mybir.dt.float32
    AF = mybir.ActivationFunctionType
    OP = mybir.AluOpType
    npc, nw = g.npc, g.nw
    total_chunks = sched["total_chunks"]
    lo_cols, hi_cols = sched["lo_cols"], sched["hi_cols"]

    def din(name, shape, dt):
        return nc.dram_tensor(name, shape, dt, kind="ExternalInput").ap()

    nodeT = din("nodeT", [HID, g.n_pad], f16)
    node_own16 = din("node_own16", [npc, HID], f16)
    node_ownT = din("node_ownT", [HID, nw * P], f16)
    W_src = din("W_src", [HID, HID], f16)
    W_dst = din("W_dst", [HID, HID], f16)
    W_out = din("W_out", [HID, HID], f16)
    W_film = din("W_film", [HID, 2 * HID], f16)
    b_film = din("b_film", [1, 2 * HID], f32)
    b_out = din("b_out", [1, HID], f32)
    task = din("task", [HID, 1], f16)
    emb8 = din("emb8", [NET, HID], f16)
    att_blk = din("att_blk", [HID, H], f16)
    lo_idx = din("lo_idx", [P, lo_cols], mybir.dt.int16)
    hi_idx = din("hi_idx", [P, hi_cols], mybir.dt.int16)
    dstr = din("dstr", [P, total_chunks], f32)
    ohetT = din("ohetT", [NET, total_chunks * P], f16)
    out = nc.dram_tensor("out", [npc, HID], f32, kind="ExternalOutput").ap()

    xsrc_tab = nc.dram_tensor("xsrc_tab", [g.n_pad, HID], f16,
                              kind="ExternalOutput" if debug else "Internal"
                              ).ap()
    if debug:
        C0 = len(sched["slabs"][0]["chunks"])
        dbg_xdst = nc.dram_tensor("dbg_xdst", [P, nw, HID], f16,
                                  kind="ExternalOutput").ap()
        dbg_resid = nc.dram_tensor("dbg_resid", [P, nw, HID], f16,
                                   kind="ExternalOutput").ap()
        dbg_xs = nc.dram_tensor("dbg_xs", [P, C0, HID], f16,
                                kind="ExternalOutput").ap()
        dbg_ohT = nc.dram_tensor("dbg_ohT", [P, C0, P], f16,
                                 kind="ExternalOutput").ap()
        dbg_comb = nc.dram_tensor("dbg_comb", [HID, C0 + 4, P], f16,
                                  kind="ExternalOutput").ap()
        dbg_rhs = nc.dram_tensor("dbg_rhs", [P, C0, 4 + HID], f16,
                                 kind="ExternalOutput").ap()
        dbg_win = nc.dram_tensor("dbg_win", [P, g.slab_w, 4 + HID], f32,
                                 kind="ExternalOutput").ap()
        dbg_oet = nc.dram_tensor("dbg_oet", [NET, C0 * P], f16,
                                 kind="ExternalOutput").ap()
        dbg_w16 = nc.dram_tensor("dbg_w16", [g.slab_w, P, 4 + HID], f32,
                                 kind="ExternalOutput").ap()
        dbg_aggn = nc.dram_tensor("dbg_aggn", [g.slab_w, P, HID], f16,
                                  kind="ExternalOutput").ap()
        dbg_y = nc.dram_tensor("dbg_y", [g.slab_w, P, HID], f32,
                               kind="ExternalOutput").ap()
        dbg_bn = nc.dram_tensor("dbg_bn", [P, g.slab_w, 2], f32,
                                kind="ExternalOutput").ap()
        dbg_sd = nc.dram_tensor("dbg_sd", [P, 2 * g.slab_w], f32,
                                kind="ExternalOutput").ap()
        dbg_yn = nc.dram_tensor("dbg_yn", [P, g.slab_w, HID], f32,
                                kind="ExternalOutput").ap()

    with tile.TileContext(nc, trace_sim=False) as tc, ExitStack() as ctx:
        cpool = ctx.enter_context(tc.tile_pool(name="consts", bufs=1))
        bpool = ctx.enter_context(tc.tile_pool(name="build", bufs=3))
        # PSUM: 8 banks of 2KB/partition, one per tile buf. Exactly 8 bufs.
        psBig = ctx.enter_context(tc.tile_pool(name="psBig", bufs=2,
                                               space="PSUM"))
        psT4 = ctx.enter_context(tc.tile_pool(name="psT4", bufs=2,
                                              space="PSUM"))
        psWin = ctx.enter_context(tc.tile_pool(name="psWin", bufs=2,
                                               space="PSUM"))
        psEx = ctx.enter_context(tc.tile_pool(name="psEx", bufs=1,
                                              space="PSUM"))
        psSm = ctx.enter_context(tc.tile_pool(name="psSm", bufs=1,
                                              space="PSUM"))
        spool = ctx.enter_context(tc.tile_pool(name="slab", bufs=2))
        gpool = ctx.enter_context(tc.tile_pool(name="grp", bufs=3))
        fpool = ctx.enter_context(tc.tile_pool(name="flush", bufs=2))

        # ---- constants ----------------------------------------------------
        ident = cpool.tile([P, P], f16)
        make_identity(nc, ident[:])
        iota16 = cpool.tile([P, P], mybir.dt.int16)
        nc.gpsimd.iota(iota16[:], pattern=[[1, P]], base=0, channel_multiplier=0)
        iota = cpool.tile([P, P], f16)
        nc.vector.tensor_copy(iota[:], iota16[:])
        ones_row = cpool.tile([1, P], f16)
        nc.vector.memset(ones_row[:], 1.0)
        eps_col = cpool.tile([P, 1], f32)
        nc.vector.memset(eps_col[:], EPS_LN)

        Ws = cpool.tile([HID, HID], f16)
        nc.sync.dma_start(Ws[:], W_src[:])
        Wd = cpool.tile([HID, HID], f16)
        nc.sync.dma_start(Wd[:], W_dst[:])
        Wo = cpool.tile([HID, HID], f16)
        nc.sync.dma_start(Wo[:], W_out[:])
        Wf = cpool.tile([HID, 2 * HID], f16)
        nc.sync.dma_start(Wf[:], W_film[:])
        emb_sb = cpool.tile([NET, HID], f16)
        nc.sync.dma_start(emb_sb[:], emb8[:])
        att_sb = cpool.tile([HID, H], f16)
        nc.sync.dma_start(att_sb[:], att_blk[:])
        task_sb = cpool.tile([HID, 1], f16)
        nc.sync.dma_start(task_sb[:], task[:])
        bfilm_sb = cpool.tile([1, 2 * HID], f32)
        nc.sync.dma_start(bfilm_sb[:], b_film[:])
        bout_sb = cpool.tile([1, HID], f32)
        nc.sync.dma_start(bout_sb[:], b_out[:])

        # residual (own node embeddings) as [P, nw, HID] f16
        resid16 = cpool.tile([P, nw, HID], f16, tag="resid")
        tail = npc - (npc // P) * P
        full_w = npc // P
        if tail:
            nc.vector.memset(resid16[:, full_w, :], 0.0)
        if full_w:
            nc.sync.dma_start(
                resid16[:, :full_w, :],
                node_own16[:full_w * P, :].rearrange("(w p) h -> p w h", p=P))
        if tail:
            nc.sync.dma_start(resid16[:tail, full_w, :],
                              node_own16[full_w * P:, :])

        # ---- FiLM ---------------------------------------------------------
        ps_f = psBig.tile([1, 2 * HID], f32, space="PSUM", tag="pcb")
        nc.tensor.matmul(out=ps_f[:], lhsT=task_sb[:], rhs=Wf[:],
                         start=True, stop=True)
        film = cpool.tile([1, 2 * HID], f32)
        nc.vector.tensor_add(film[:], ps_f[:], bfilm_sb[:])
        gam_t = cpool.tile([1, HID], f32)
        nc.scalar.activation(gam_t[:], film[:, :HID], AF.Tanh)
        gam16 = cpool.tile([1, HID], f16)
        nc.vector.tensor_scalar(gam16[:], gam_t[:], 0.5, 1.0, OP.mult, OP.add)
        tmpb = cpool.tile([1, HID], f32)
        nc.vector.tensor_mul(tmpb[:], bout_sb[:], gam16[:])
        beta16 = cpool.tile([1, HID], f16)
        nc.vector.tensor_add(beta16[:], tmpb[:], film[:, HID:])
        ps_g = psBig.tile([P, HID], f32, space="PSUM", tag="pcb")
        nc.tensor.matmul(out=ps_g[:], lhsT=ones_row[:], rhs=gam16[:],
                         start=True, stop=True)
        gam_rep = cpool.tile([P, HID], f16)
        nc.vector.tensor_copy(gam_rep[:], ps_g[:])
        Wosc = cpool.tile([HID, HID], f16)
        nc.vector.tensor_mul(Wosc[:], Wo[:], gam_rep[:])

        if not sched["skip_norm"]:
            nw_dr = din("normw", [1, HID], f32)
            nb_dr = din("normb", [1, HID], f32)
            nw_sb = cpool.tile([1, HID], f32)
            nc.sync.dma_start(nw_sb[:], nw_dr[:])
            nb_sb = cpool.tile([1, HID], f32)
            nc.sync.dma_start(nb_sb[:], nb_dr[:])
            ones32 = cpool.tile([1, P], f32)
            nc.vector.memset(ones32[:], 1.0)
            ps_w = psBig.tile([P, HID], f32, space="PSUM", tag="pcb")
            nc.tensor.matmul(out=ps_w[:], lhsT=ones32[:], rhs=nw_sb[:],
                             start=True, stop=True)
            w_rep = cpool.tile([P, HID], f32)
            nc.vector.tensor_copy(w_rep[:], ps_w[:])
            ps_b = psBig.tile([P, HID], f32, space="PSUM", tag="pcb")
            nc.tensor.matmul(out=ps_b[:], lhsT=ones32[:], rhs=nb_sb[:],
                             start=True, stop=True)
            b_rep = cpool.tile([P, HID], f32)
            nc.vector.tensor_copy(b_rep[:], ps_b[:])

        # ---- xdst table (SBUF-resident) -----------------------------------
        xdst_sb = cpool.tile([P, nw, HID], f16, tag="xdst")
        for gl in range(0, nw, 16):
            gln = min(16, nw - gl)
            not_t = bpool.tile([HID, 16 * P], f16, tag="not")
            nc.sync.dma_start(not_t[:, :gln * P],
                              node_ownT[:, gl * P:(gl + gln) * P])
            for gw in range(gl, gl + gln, 4):
                gn = min(4, gl + gln - gw)
                psX = psBig.tile([P, 4, HID], f32, space="PSUM", tag="pcb")
                for k in range(gn):
                    nc.tensor.matmul(out=psX[:, k, :],
                                     lhsT=not_t[:, (gw - gl + k) * P:
                                                (gw - gl + k + 1) * P],
                                     rhs=Wd[:],
                                     start=True, stop=True,
                                     skip_group_check=True)
                nc.scalar.activation(xdst_sb[:, gw:gw + gn, :], psX[:, :gn, :],
                                     AF.Prelu, alpha=1.0)

        # ---- xsrc gather table --------------------------------------------
        # 2048-node iterations: one big nodeT load (SP) + one big table write
        # (Act) each -- HWDGE charges ~0.6us per DMA instruction, so few big
        # DMAs beat many small ones. PSUM->SBUF copies alternate DVE/Act.
        starts = list(range(0, g.n_pad, 2048))  # last block may be partial
        nts = []
        for i, st in enumerate(starts):
            wdt = min(2048, g.n_pad - st)
            nt = bpool.tile([HID, 2048], f16, tag="nt", name=f"nt{i}",
                            bufs=3)
            nc.sync.dma_start(nt[:, :wdt], nodeT[:, st:st + wdt])
            nts.append(nt)
        for i, st in enumerate(starts):
            wdt = min(2048, g.n_pad - st)
            nq = wdt // 512
            nt = nts[i]
            xt = bpool.tile([P, 4, 4, HID], f16, tag="xt", bufs=3)
            for q in range(nq):
                ps = psBig.tile([P, 4, HID], f32, space="PSUM", tag="pcb")
                for j in range(4):
                    nc.tensor.matmul(out=ps[:, j, :],
                                     lhsT=nt[:, q * 512 + j * P:
                                             q * 512 + (j + 1) * P],
                                     rhs=Ws[:],
                                     start=True, stop=True,
                                     skip_group_check=True)
                if q % 2 == 0:
                    nc.vector.tensor_copy(xt[:, q, :, :], ps[:])
                else:
                    nc.scalar.activation(xt[:, q, :, :], ps[:],
                                         AF.Prelu, alpha=1.0)
            # dram row st + q*512 + p*4 + s  <-  xt[p, q, s, :]
            nc.scalar.dma_start(
                xsrc_tab[st:st + wdt, :]
                .rearrange("(q p s) h -> p q s h", p=P, s=4),
                xt[:, :nq, :, :])

        if debug:
            nc.sync.dma_start(dbg_xdst[:], xdst_sb[:])
            nc.sync.dma_start(dbg_resid[:], resid16[:])

        # ---- idx + dstr staging --------------------------------------------
        lo_sb = cpool.tile([P, lo_cols], mybir.dt.int16, tag="loidx")
        nc.sync.dma_start(lo_sb[:], lo_idx[:])
        hi_sb = cpool.tile([P, hi_cols], mybir.dt.int16, tag="hiidx")
        nc.sync.dma_start(hi_sb[:], hi_idx[:])
        dstr_sb = cpool.tile([P, total_chunks], f32, tag="dstr")
        nc.sync.dma_start(dstr_sb[:], dstr[:])

        off16 = {0: 0, 1: 0}
        qn = [0]

        def nextq():
            qn[0] = (qn[0] + 1) % 4
            return qn[0]

        # ---- edge slabs ----------------------------------------------------
        # Software-pipelined emission: each slab's input loads and one-hot
        # generation ("front") are emitted two slabs ahead of its compute +
        # flush ("back"), so the in-order per-engine sequencers never park a
        # next-slab load behind a previous slab's dependency waits.
        fronts = {}
        wins = {}

        def emit_front(s):
            sl = sched["slabs"][s]
            chunks = sl["chunks"]
            C = len(chunks)
            c0 = sl["chunk0"]

            xs_t = spool.tile([P, C, HID], f16, tag="xs", bufs=3,
                              name=f"xs{s}")
            for h in (0, 1):
                base = 0 if h == 0 else g.split
                idx_sb = lo_sb if h == 0 else hi_sb
                for (slot_off, n) in sl["calls"][h]:
                    if n == 0:
                        continue
                    nc.gpsimd.dma_gather(
                        out_ap=xs_t[:, slot_off:slot_off + n // P, :],
                        in_ap=xsrc_tab[base:g.n_pad, :],
                        idxs_ap=idx_sb[:, off16[h]:off16[h] + n // 16],
                        num_idxs=n, num_idxs_reg=n, elem_size=HID,
                        single_packet=(n <= 1024), queue_num=nextq(),
                    )
                    off16[h] += n // 16

            oet_t = spool.tile([NET, C * P], f16, tag="oet", bufs=3,
                               name=f"oet{s}")
            nc.sync.dma_start(oet_t[:], ohetT[:, c0 * P:(c0 + C) * P])

            oh_t = spool.tile([P, C, P], f16, tag="oh", bufs=3,
                              name=f"oh{s}")
            for ci in range(C):
                eng = nc.gpsimd if ci % 4 == 3 else nc.vector
                eng.tensor_scalar(
                    oh_t[:, ci, :], iota[:], dstr_sb[:, c0 + ci:c0 + ci + 1],
                    None, OP.is_equal)
            fronts[s] = (xs_t, oet_t, oh_t)

        def emit_back(s):
            sl = sched["slabs"][s]
            ws = sl["windows"]
            nwin = len(ws)
            chunks = sl["chunks"]
            C = len(chunks)
            c0 = sl["chunk0"]
            xs_t, oet_t, oh_t = fronts.pop(s)

            ohT_t = spool.tile([P, C, P], f16, tag="ohT", name=f"ohT{s}")
            rhs_t = spool.tile([P, C, 4 + HID], f16, tag="rhs",
                               name=f"rhs{s}")
            ex_ps = psEx.tile([P, C, H], f32, space="PSUM", tag="ex")

            ngrp = (C + 3) // 4
            for gi in range(ngrp):
                gn = min(4, C - 4 * gi)
                cs0 = 4 * gi
                # -- ohT for this group
                if True:
                    psT = psT4.tile([P, 4, P], f16, space="PSUM", tag="psT")
                    for k in range(gn):
                        nc.tensor.transpose(out=psT[:, k, :],
                                            in_=oh_t[:, cs0 + k, :],
                                            identity=ident[:])
                    if gi % 8 == 6:
                        nc.scalar.activation(ohT_t[:, cs0:cs0 + gn, :],
                                             psT[:, :gn, :], AF.Prelu,
                                             alpha=1.0)
                    else:
                        nc.vector.tensor_copy(ohT_t[:, cs0:cs0 + gn, :],
                                              psT[:, :gn, :])
                # -- combT accumulation in PSUM
                pcb = psBig.tile([HID, 4, P], f32, space="PSUM", tag="pcb")
                for k in range(gn):
                    ci = cs0 + k
                    wl = chunks[ci][0]
                    nc.tensor.matmul(out=pcb[:, k, :],
                                     lhsT=xdst_sb[:, ws[wl], :],
                                     rhs=ohT_t[:, ci, :],
                                     start=True, stop=False,
                                     skip_group_check=True)
                    nc.tensor.matmul(out=pcb[:, k, :], lhsT=emb_sb[:],
                                     rhs=oet_t[:, ci * P:(ci + 1) * P],
                                     start=False, stop=False,
                                     skip_group_check=True)
                    nc.tensor.matmul(out=pcb[:, k, :], lhsT=xs_t[:, ci, :],
                                     rhs=ident[:], start=False, stop=True,
                                     skip_group_check=True)
                combT = gpool.tile([HID, 4, P], f16, tag="comb")
                nc.scalar.activation(combT[:, :gn, :], pcb[:, :gn, :],
                                     AF.Prelu, alpha=0.2)
                if debug and s == 0:
                    nc.sync.dma_start(dbg_comb[:, cs0:cs0 + gn, :],
                                      combT[:, :gn, :])
                for k in range(gn):
                    ci = cs0 + k
                    nc.tensor.matmul(out=ex_ps[:, ci, :], lhsT=combT[:, k, :],
                                     rhs=att_sb[:], start=True, stop=True,
                                     skip_group_check=True)

            for g8 in range(0, C, 4):
                gn8 = min(4, C - g8)
                nc.scalar.activation(rhs_t[:, g8:g8 + gn8, 0:4],
                                     ex_ps[:, g8:g8 + gn8, :], AF.Exp)

            for ci in range(C):
                eng = (nc.vector if (ci % g.wmul_pool_mod
                                     == g.wmul_pool_mod - 1) else nc.gpsimd)
                eng.tensor_mul(
                    rhs_t[:, ci, 4:].rearrange("p (h d) -> p h d", h=H),
                    xs_t[:, ci, :].rearrange("p (h d) -> p h d", h=H),
                    rhs_t[:, ci, 0:4].unsqueeze(2).broadcast_to([P, H, HD]))

            # -- scatter into window accumulators
            win_t = psWin.tile([P, g.slab_w, 4 + HID], f32, space="PSUM",
                               tag="win", name=f"win{s}")
            n_per_win = [0] * nwin
            for (wl, h, slot) in chunks:
                n_per_win[wl] += 1
            # window-major scatter order: accumulation groups sharing a
            # PSUM bank must not interleave on hardware
            seen = [0] * nwin
            order = sorted(range(C), key=lambda ci: chunks[ci][0])
            for ci in order:
                wl = chunks[ci][0]
                first = seen[wl] == 0
                last = seen[wl] == n_per_win[wl] - 1
                seen[wl] += 1
                nc.tensor.matmul(out=win_t[:, wl, :], lhsT=oh_t[:, ci, :],
                                 rhs=rhs_t[:, ci, :], start=first, stop=last,
                                 skip_group_check=True)
            if debug and s == 0:
                nc.sync.dma_start(dbg_xs[:], xs_t[:])
                nc.sync.dma_start(dbg_ohT[:], ohT_t[:])
                nc.sync.dma_start(dbg_rhs[:], rhs_t[:])
                nc.sync.dma_start(dbg_oet[:], oet_t[:])
            wins[s] = (win_t, n_per_win)

        def emit_flush(s):
            # ---- flush windows (emitted one slab late so these dependency-
            # waiting ops never park in front of the next slab's work on the
            # in-order per-engine sequencers) --------------------------------
            sl = sched["slabs"][s]
            ws = sl["windows"]
            nwin = len(ws)
            win_t, n_per_win = wins.pop(s)
            if debug and s == 0:
                for _wl in range(len(ws)):
                    dbgw = fpool.tile([P, 4 + HID], f32, tag="yn")
                    nc.vector.tensor_copy(dbgw[:], win_t[:, _wl, :])
                    nc.sync.dma_start(dbg_win[:, _wl, :], dbgw[:])
            bn_sl = fpool.tile([P, nwin, 2], f32, tag="bnsl", name=f"bns{s}")
            y_l = []
            for wl, w in enumerate(ws):
                assert n_per_win[wl] > 0
                # f32: raw exp-sums can exceed the f16 max before the
                # normalization divide
                win16 = fpool.tile([P, 4 + HID], f32, tag="win16")
                nc.scalar.activation(win16[:], win_t[:, wl, :], AF.Prelu,
                                     alpha=1.0)
                sums = fpool.tile([P, 4], f32, tag="sums")
                nc.vector.tensor_scalar(sums[:], win16[:, 0:4], 1e-12, None,
                                        OP.max)
                rec = fpool.tile([P, 4], f32, tag="rec")
                nc.vector.reciprocal(rec[:], sums[:])
                if debug and s == 0:
                    nc.sync.dma_start(dbg_w16[wl], win16[:])
                aggn = fpool.tile([P, HID], f16, tag="aggn")
                nc.vector.tensor_mul(
                    aggn[:].rearrange("p (h d) -> p h d", h=H),
                    win16[:, 4:].rearrange("p (h d) -> p h d", h=H),
                    rec[:].unsqueeze(2).broadcast_to([P, H, HD]))
                if debug and s == 0:
                    nc.sync.dma_start(dbg_aggn[wl], aggn[:])
                psTf = psSm.tile([P, P], f16, space="PSUM", tag="pt")
                nc.tensor.transpose(out=psTf[:], in_=aggn[:], identity=ident[:])
                aggT = fpool.tile([HID, P], f16, tag="aggT")
                if wl % 2 == 0:
                    nc.scalar.activation(aggT[:], psTf[:], AF.Prelu, alpha=1.0)
                else:
                    nc.vector.tensor_copy(aggT[:], psTf[:])
                po = psSm.tile([P, HID], f32, space="PSUM", tag="pt")
                nc.tensor.matmul(out=po[:], lhsT=aggT[:], rhs=Wosc[:],
                                 start=True, stop=False,
                                 skip_group_check=True)
                nc.tensor.matmul(out=po[:], lhsT=ones_row[:], rhs=beta16[:],
                                 start=False, stop=True,
                                 skip_group_check=True)
                y = fpool.tile([P, HID], f32, tag="y", name=f"y{s}_{wl}",
                               bufs=g.slab_w + 1)
                nc.vector.tensor_add(y[:], po[:], resid16[:, w, :])
                y_l.append(y)
                if debug and s == 0:
                    nc.sync.dma_start(dbg_y[wl], y[:])
                bnst = fpool.tile([P, 6], f32, tag="bnst")
                nc.vector.bn_stats(bnst[:], y[:])
                nc.vector.bn_aggr(bn_sl[:, wl, :], bnst[:])
            sd = fpool.tile([P, nwin], f32, tag="sd", name=f"sd{s}")
            nc.scalar.activation(sd[:], bn_sl[:, :, 1], AF.Sqrt,
                                 bias=eps_col[:])
            rstd = fpool.tile([P, nwin], f32, tag="rstd", name=f"rs{s}")
            nc.vector.reciprocal(rstd[:], sd[:])
            if debug and s == 0:
                nc.sync.dma_start(dbg_bn[:], bn_sl[:])
                nc.sync.dma_start(dbg_sd[:, 0:nwin], sd[:])
                nc.sync.dma_start(dbg_sd[:, g.slab_w:g.slab_w + nwin],
                                  rstd[:])
            yn32 = fpool.tile([P, nwin, HID], f32, tag="yn", name=f"yn{s}")
            for wl, w in enumerate(ws):
                nc.vector.scalar_tensor_tensor(
                    yn32[:, wl, :], y_l[wl][:], bn_sl[:, wl, 0:1],
                    rstd[:, wl:wl + 1].broadcast_to([P, HID]),
                    OP.subtract, OP.mult)
                if not sched["skip_norm"]:
                    nc.vector.tensor_mul(yn32[:, wl, :], yn32[:, wl, :],
                                         w_rep[:])
                    nc.vector.tensor_add(yn32[:, wl, :], yn32[:, wl, :],
                                         b_rep[:])
            if debug and s == 0:
                nc.sync.dma_start(dbg_yn[:], yn32[:])
            nfull = sum(1 for w in ws if npc - w * P >= P)
            if nfull:
                nc.sync.dma_start(
                    out[ws[0] * P:ws[0] * P + nfull * P, :]
                    .rearrange("(w p) h -> p w h", p=P),
                    yn32[:, :nfull, :])
            for wl, w in enumerate(ws):
                rows = npc - w * P
                if rows < P:
                    nc.sync.dma_start(out[w * P:w * P + rows, :],
                                      yn32[:rows, wl, :])

        nslab = len(sched["slabs"])
        emit_front(0)
        if nslab > 1:
            emit_front(1)
        for s in range(nslab):
            if s + 2 < nslab:
                emit_front(s + 2)
            if s >= 1:
                emit_flush(s - 1)
            emit_back(s)
        emit_flush(nslab - 1)

    nc.compile()
    return nc


# ---------------------------------------------------------------------------
# Full-input entry point: shard, compile (cached), run SPMD on 8 cores,
# gather the output shards.
# ---------------------------------------------------------------------------
_CACHE = {}


def kernel(**inputs):
    N = int(np.asarray(inputs["node_embeddings"]).shape[0])
    n_cores = 8
    g = Geo(N=N, n_cores=n_cores)

    sched, in_maps = host_prep(g, **{k: np.asarray(v) for k, v in inputs.items()})

    key = (N, sched["total_chunks"], tuple(int(x) for x in sched["caps"].ravel()),
           sched["skip_norm"])
    if key not in _CACHE:
        _CACHE[key] = build_program(g, sched)
    nc = _CACHE[key]

    from concourse.bass_utils import run_bass_kernel_spmd
    res = run_bass_kernel_spmd(nc, in_maps, core_ids=list(range(n_cores)))
    out = np.concatenate([res.results[c]["out"] for c in range(n_cores)], axis=0)
    return out.astype(np.float32)


# revision 84
# speedup vs baseline: 1.0218x; 1.0218x over previous
"""GATv2 graph layer Bass kernel for TRN2 (SPMD across 8 NeuronCores, no
collectives).

Design (v2): edges sorted by destination node and sharded across cores by dst
range. Each core builds ONE fp16 gather table in DRAM (xsrc = node_emb @
W_src, split in two halves so int16 gather indices fit), with rows remapped so
table writes use 1KB descriptors. Destination-side features are NOT gathered
from DRAM: x_dst for the core's own nodes lives in SBUF ([P, nw, HID] f16),
and per edge chunk (128 edges, one 128-node dst window) the dst contribution
is gathered by the TensorEngine via a transposed one-hot:

  oh[e, d]  = (dstr[e] == d)                       (DVE tensor_scalar)
  ohT[d, e] = transpose(oh)                        (PE transpose or XBAR DMA)
  psum_combT[hid, e] = xdst_win^T @ ohT            (lhsT=xdst_win)
                     + emb8^T @ ohetT              (host one-hot over 8 types)
                     + xs_chunk^T                  (lhsT=xs, rhs=identity)
  combT16 = Prelu(psum_combT, alpha=0.2)           (Act: fused lrelu + copy)
  ex_ps[e, h] = combT16^T @ att_blk                (PE)
  ex16 = Exp(ex_ps)                                (Act, 8-chunk groups)
  wgt = xs * bcast(ex16)                           (DVE/Pool)
  win_ps[dst, 4+HID] += oh^T @ [ex16 | wgt]        (PE scatter)

Window flush: agg = sum(ex*xs)/sum(ex), @W_out (gamma-scaled) + beta_eff,
residual, LayerNorm via bn_stats/bn_aggr, DMA out. No max-subtraction in the
softmax: logits are bounded so exp stays finite in f32 (matches reference to
~1e-4).
"""
import numpy as np
from contextlib import ExitStack
from dataclasses import dataclass

import concourse.bass as bass
import concourse.tile as tile
from concourse import bacc, mybir
from concourse.masks import make_identity

P = 128
HID = 128
H = 4
HD = 32
NET = 8
EPS_LN = 1e-5
MAXCALL = 4096  # max idxs per dma_gather call
DEAD = -5.0     # dst_rel for padding slots (matches no one-hot column)


@dataclass
class Geo:
    N: int
    n_cores: int
    slab_w: int = 3     # windows per slab
    dma_q: int = 0      # unused (PE transpose path)
    wmul_pool_mod: int = 2  # every k-th chunk's weight-mul runs on DVE

    @property
    def npc(self):
        return self.N // self.n_cores

    @property
    def nw(self):
        return (self.npc + P - 1) // P

    @property
    def nslab(self):
        return (self.nw + self.slab_w - 1) // self.slab_w

    @property
    def n_pad(self):   # padded node count (512 blocks)
        return ((self.N + 511) // 512) * 512

    @property
    def split(self):   # lo/hi table split on a 512 block boundary
        return (self.n_pad // 1024) * 512


def wrap_idx(idx, cols):
    n = idx.shape[0]
    assert n % 16 == 0
    w = np.zeros((P, cols), dtype=np.int16)
    if n:
        t16 = idx.reshape(n // 16, 16).T
        for g in range(8):
            w[g * 16:(g + 1) * 16, :n // 16] = t16
    return w


def remap_row(r):
    """Table row remap so device-side table writes are 1KB/partition:
    original row i*512 + s*128 + p is stored at i*512 + p*4 + s."""
    i, rem = r // 512, r % 512
    return i * 512 + (rem % 128) * 4 + rem // 128


def host_prep(g: Geo, node_embeddings, edge_index, edge_type, task_embedding,
              W_src, b_src, W_dst, b_dst, edge_emb, att,
              W_out, b_out, norm_w, norm_b, W_film, b_film):
    """Returns (sched, in_maps). Pure index work + tiny constant folding;
    all O(N*HID) / O(E*HID) float math runs on device."""
    src = np.asarray(edge_index[0], dtype=np.int64)
    dst = np.asarray(edge_index[1], dtype=np.int64)
    et = np.asarray(edge_type, dtype=np.int64)
    npc, split = g.npc, g.split

    order = np.argsort(dst, kind="stable")
    src, dst, et = src[order], dst[order], et[order]
    core_of = dst // npc

    buckets = {}
    for c in range(g.n_cores):
        m = core_of == c
        cs, cd, ce = src[m], dst[m] - c * npc, et[m]
        for w in range(g.nw):
            wm = (cd // P) == w
            ws_, wd, we = cs[wm], cd[wm] - w * P, ce[wm]
            lo = ws_ < split
            buckets[(c, w, 0)] = (remap_row(ws_[lo]), wd[lo], we[lo])
            buckets[(c, w, 1)] = (remap_row(ws_[~lo] - split), wd[~lo], we[~lo])

    caps = np.zeros((g.nw, 2), dtype=np.int64)
    for w in range(g.nw):
        for h in range(2):
            mx = max(len(buckets[(c, w, h)][0]) for c in range(g.n_cores))
            caps[w, h] = (mx + P - 1) // P

    # ---- schedule ---------------------------------------------------------
    sched_slabs = []
    total_chunks = 0
    for s in range(g.nslab):
        ws = list(range(s * g.slab_w, min((s + 1) * g.slab_w, g.nw)))
        chunks = []            # (win_local, half, slot)
        calls = {0: [], 1: []}
        slot = 0
        for h in (0, 1):
            run = 0
            run_start = slot
            for w in ws:
                for _ in range(caps[w, h]):
                    chunks.append((w - ws[0], h, slot))
                    slot += 1
                    run += P
                    if run == MAXCALL:
                        calls[h].append((run_start, run))
                        run, run_start = 0, slot
            if run:
                calls[h].append((run_start, run))
        sched_slabs.append(dict(windows=ws, chunks=chunks, calls=calls,
                                chunk0=total_chunks))
        total_chunks += len(chunks)

    lo_cols = max(16, sum(n for sl in sched_slabs
                          for (_, n) in sl["calls"][0]) // 16)
    hi_cols = max(16, sum(n for sl in sched_slabs
                          for (_, n) in sl["calls"][1]) // 16)

    # ---- shared constants -------------------------------------------------
    nodeT = np.zeros((HID, g.n_pad), dtype=np.float16)
    nodeT[:, :g.N] = np.asarray(node_embeddings, np.float32).T.astype(np.float16)
    emb_eff = (np.asarray(edge_emb, np.float64)
               + np.asarray(b_src, np.float64)[None, :]
               + np.asarray(b_dst, np.float64)[None, :]).astype(np.float16)
    att_blk = np.zeros((HID, H), dtype=np.float16)
    for h in range(H):
        att_blk[h * HD:(h + 1) * HD, h] = np.asarray(att, np.float32)[h]

    consts = dict(
        nodeT=nodeT,
        W_src=np.asarray(W_src, np.float32).astype(np.float16),
        W_dst=np.asarray(W_dst, np.float32).astype(np.float16),
        W_out=np.asarray(W_out, np.float32).astype(np.float16),
        W_film=np.asarray(W_film, np.float32).astype(np.float16),
        b_film=np.asarray(b_film, np.float32).reshape(1, 2 * HID),
        b_out=np.asarray(b_out, np.float32).reshape(1, HID),
        task=np.asarray(task_embedding, np.float32).reshape(HID, 1)
            .astype(np.float16),
        emb8=emb_eff,                       # [8, HID] f16
        att_blk=att_blk,
    )
    skip_norm = bool(np.all(np.asarray(norm_w) == 1.0)
                     and np.all(np.asarray(norm_b) == 0.0))
    if not skip_norm:
        consts["normw"] = np.asarray(norm_w, np.float32).reshape(1, HID)
        consts["normb"] = np.asarray(norm_b, np.float32).reshape(1, HID)

    node_f16 = np.asarray(node_embeddings, np.float32).astype(np.float16)

    # ---- per-core arrays --------------------------------------------------
    in_maps = []
    for c in range(g.n_cores):
        lo_l, hi_l = [], []
        dstr = np.full((P, total_chunks), DEAD, dtype=np.float32)
        oet = np.zeros((NET, total_chunks * P), dtype=np.float16)
        ci = 0
        for sl in sched_slabs:
            ws0 = sl["windows"][0]
            per_half = {0: [], 1: []}
            nth = {}
            for (wl, h, slot) in sl["chunks"]:
                w = ws0 + wl
                es, ed, ee = buckets[(c, w, h)]
                k = nth.get((wl, h), 0)
                nth[(wl, h)] = k + 1
                sl_src = np.zeros(P, dtype=np.int64)
                n = min(P, max(0, len(es) - k * P))
                if n > 0:
                    sl_src[:n] = es[k * P:k * P + n]
                    dstr[:n, ci] = ed[k * P:k * P + n]
                    oet[ee[k * P:k * P + n], ci * P + np.arange(n)] = 1.0
                per_half[h].append(sl_src)
                ci += 1
            lo_l.extend(per_half[0])
            hi_l.extend(per_half[1])
        lo_i = (np.concatenate(lo_l) if lo_l else np.zeros(0, np.int64))
        hi_i = (np.concatenate(hi_l) if hi_l else np.zeros(0, np.int64))
        assert lo_i.max(initial=0) < g.split <= 32767
        assert hi_i.max(initial=0) < g.n_pad - g.split <= 32768

        own = node_f16[c * npc:(c + 1) * npc]          # [npc, HID] f16
        ownT = np.zeros((HID, g.nw * P), dtype=np.float16)
        ownT[:, :npc] = own.T

        m = dict(consts)
        m["node_own16"] = np.ascontiguousarray(own)
        m["node_ownT"] = ownT
        m["lo_idx"] = wrap_idx(lo_i.astype(np.int16), lo_cols)
        m["hi_idx"] = wrap_idx(hi_i.astype(np.int16), hi_cols)
        m["dstr"] = dstr
        m["ohetT"] = oet
        in_maps.append(m)

    sched = dict(slabs=sched_slabs, caps=caps, total_chunks=total_chunks,
                 lo_cols=lo_cols, hi_cols=hi_cols, skip_norm=skip_norm)
    return sched, in_maps


def build_program(g: Geo, sched, debug=False):
    nc = bacc.Bacc("TRN2", target_bir_lowering=False, debug=False,
                   num_devices=g.n_cores, num_swdge_queues=4)
    f16, f32 = mybir.dt.float16, mybir.dt.float32
    AF = mybir.ActivationFunctionType
    OP = mybir.AluOpType
    npc, nw = g.npc, g.nw
    total_chunks = sched["total_chunks"]
    lo_cols, hi_cols = sched["lo_cols"], sched["hi_cols"]

    def din(name, shape, dt):
        return nc.dram_tensor(name, shape, dt, kind="ExternalInput").ap()

    nodeT = din("nodeT", [HID, g.n_pad], f16)
    node_own16 = din("node_own16", [npc, HID], f16)
    node_ownT = din("node_ownT", [HID, nw * P], f16)
    W_src = din("W_src", [HID, HID], f16)
    W_dst = din("W_dst", [HID, HID], f16)
    W_out = din("W_out", [HID, HID], f16)
    W_film = din("W_film", [HID, 2 * HID], f16)
    b_film = din("b_film", [1, 2 * HID], f32)
    b_out = din("b_out", [1, HID], f32)
    task = din("task", [HID, 1], f16)
    emb8 = din("emb8", [NET, HID], f16)
    att_blk = din("att_blk", [HID, H], f16)
    lo_idx = din("lo_idx", [P, lo_cols], mybir.dt.int16)
    hi_idx = din("hi_idx", [P, hi_cols], mybir.dt.int16)
    dstr = din("dstr", [P, total_chunks], f32)
    ohetT = din("ohetT", [NET, total_chunks * P], f16)
    out = nc.dram_tensor("out", [npc, HID], f32, kind="ExternalOutput").ap()

    xsrc_tab = nc.dram_tensor("xsrc_tab", [g.n_pad, HID], f16,
                              kind="ExternalOutput" if debug else "Internal"
                              ).ap()
    if debug:
        C0 = len(sched["slabs"][0]["chunks"])
        dbg_xdst = nc.dram_tensor("dbg_xdst", [P, nw, HID], f16,
                                  kind="ExternalOutput").ap()
        dbg_resid = nc.dram_tensor("dbg_resid", [P, nw, HID], f16,
                                   kind="ExternalOutput").ap()
        dbg_xs = nc.dram_tensor("dbg_xs", [P, C0, HID], f16,
                                kind="ExternalOutput").ap()
        dbg_ohT = nc.dram_tensor("dbg_ohT", [P, C0, P], f16,
                                 kind="ExternalOutput").ap()
        dbg_comb = nc.dram_tensor("dbg_comb", [HID, C0 + 4, P], f16,
                                  kind="ExternalOutput").ap()
        dbg_rhs = nc.dram_tensor("dbg_rhs", [P, C0, 4 + HID], f16,
                                 kind="ExternalOutput").ap()
        dbg_win = nc.dram_tensor("dbg_win", [P, g.slab_w, 4 + HID], f32,
                                 kind="ExternalOutput").ap()
        dbg_oet = nc.dram_tensor("dbg_oet", [NET, C0 * P], f16,
                                 kind="ExternalOutput").ap()
        dbg_w16 = nc.dram_tensor("dbg_w16", [g.slab_w, P, 4 + HID], f32,
                                 kind="ExternalOutput").ap()
        dbg_aggn = nc.dram_tensor("dbg_aggn", [g.slab_w, P, HID], f16,
                                  kind="ExternalOutput").ap()
        dbg_y = nc.dram_tensor("dbg_y", [g.slab_w, P, HID], f32,
                               kind="ExternalOutput").ap()
        dbg_bn = nc.dram_tensor("dbg_bn", [P, g.slab_w, 2], f32,
                                kind="ExternalOutput").ap()
        dbg_sd = nc.dram_tensor("dbg_sd", [P, 2 * g.slab_w], f32,
                                kind="ExternalOutput").ap()
        dbg_yn = nc.dram_tensor("dbg_yn", [P, g.slab_w, HID], f32,
                                kind="ExternalOutput").ap()

    with tile.TileContext(nc, trace_sim=False) as tc, ExitStack() as ctx:
        cpool = ctx.enter_context(tc.tile_pool(name="consts", bufs=1))
        bpool = ctx.enter_context(tc.tile_pool(name="build", bufs=3))
        # PSUM: 8 banks of 2KB/partition, one per tile buf. Exactly 8 bufs.
        psBig = ctx.enter_context(tc.tile_pool(name="psBig", bufs=2,
                                               space="PSUM"))
        psT4 = ctx.enter_context(tc.tile_pool(name="psT4", bufs=2,
                                              space="PSUM"))
        psWin = ctx.enter_context(tc.tile_pool(name="psWin", bufs=2,
                                               space="PSUM"))
        psEx = ctx.enter_context(tc.tile_pool(name="psEx", bufs=1,
                                              space="PSUM"))
        psSm = ctx.enter_context(tc.tile_pool(name="psSm", bufs=1,
                                              space="PSUM"))
        spool = ctx.enter_context(tc.tile_pool(name="slab", bufs=2))
        gpool = ctx.enter_context(tc.tile_pool(name="grp", bufs=3))
        fpool = ctx.enter_context(tc.tile_pool(name="flush", bufs=2))

        # ---- constants ----------------------------------------------------
        ident = cpool.tile([P, P], f16)
        make_identity(nc, ident[:])
        iota16 = cpool.tile([P, P], mybir.dt.int16)
        nc.gpsimd.iota(iota16[:], pattern=[[1, P]], base=0, channel_multiplier=0)
        iota = cpool.tile([P, P], f16)
        nc.vector.tensor_copy(iota[:], iota16[:])
        ones_row = cpool.tile([1, P], f16)
        nc.vector.memset(ones_row[:], 1.0)
        eps_col = cpool.tile([P, 1], f32)
        nc.vector.memset(eps_col[:], EPS_LN)

        Ws = cpool.tile([HID, HID], f16)
        nc.sync.dma_start(Ws[:], W_src[:])
        Wd = cpool.tile([HID, HID], f16)
        nc.sync.dma_start(Wd[:], W_dst[:])
        Wo = cpool.tile([HID, HID], f16)
        nc.sync.dma_start(Wo[:], W_out[:])
        Wf = cpool.tile([HID, 2 * HID], f16)
        nc.sync.dma_start(Wf[:], W_film[:])
        emb_sb = cpool.tile([NET, HID], f16)
        nc.sync.dma_start(emb_sb[:], emb8[:])
        att_sb = cpool.tile([HID, H], f16)
        nc.sync.dma_start(att_sb[:], att_blk[:])
        task_sb = cpool.tile([HID, 1], f16)
        nc.sync.dma_start(task_sb[:], task[:])
        bfilm_sb = cpool.tile([1, 2 * HID], f32)
        nc.sync.dma_start(bfilm_sb[:], b_film[:])
        bout_sb = cpool.tile([1, HID], f32)
        nc.sync.dma_start(bout_sb[:], b_out[:])

        # residual (own node embeddings) as [P, nw, HID] f16
        resid16 = cpool.tile([P, nw, HID], f16, tag="resid")
        tail = npc - (npc // P) * P
        full_w = npc // P
        if tail:
            nc.vector.memset(resid16[:, full_w, :], 0.0)
        if full_w:
            nc.sync.dma_start(
                resid16[:, :full_w, :],
                node_own16[:full_w * P, :].rearrange("(w p) h -> p w h", p=P))
        if tail:
            nc.sync.dma_start(resid16[:tail, full_w, :],
                              node_own16[full_w * P:, :])

        # ---- FiLM ---------------------------------------------------------
        ps_f = psBig.tile([1, 2 * HID], f32, space="PSUM", tag="pcb")
        nc.tensor.matmul(out=ps_f[:], lhsT=task_sb[:], rhs=Wf[:],
                         start=True, stop=True)
        film = cpool.tile([1, 2 * HID], f32)
        nc.vector.tensor_add(film[:], ps_f[:], bfilm_sb[:])
        gam_t = cpool.tile([1, HID], f32)
        nc.scalar.activation(gam_t[:], film[:, :HID], AF.Tanh)
        gam16 = cpool.tile([1, HID], f16)
        nc.vector.tensor_scalar(gam16[:], gam_t[:], 0.5, 1.0, OP.mult, OP.add)
        tmpb = cpool.tile([1, HID], f32)
        nc.vector.tensor_mul(tmpb[:], bout_sb[:], gam16[:])
        beta16 = cpool.tile([1, HID], f16)
        nc.vector.tensor_add(beta16[:], tmpb[:], film[:, HID:])
        ps_g = psBig.tile([P, HID], f32, space="PSUM", tag="pcb")
        nc.tensor.matmul(out=ps_g[:], lhsT=ones_row[:], rhs=gam16[:],
                         start=True, stop=True)
        gam_rep = cpool.tile([P, HID], f16)
        nc.vector.tensor_copy(gam_rep[:], ps_g[:])
        Wosc = cpool.tile([HID, HID], f16)
        nc.vector.tensor_mul(Wosc[:], Wo[:], gam_rep[:])

        if not sched["skip_norm"]:
            nw_dr = din("normw", [1, HID], f32)
            nb_dr = din("normb", [1, HID], f32)
            nw_sb = cpool.tile([1, HID], f32)
            nc.sync.dma_start(nw_sb[:], nw_dr[:])
            nb_sb = cpool.tile([1, HID], f32)
            nc.sync.dma_start(nb_sb[:], nb_dr[:])
            ones32 = cpool.tile([1, P], f32)
            nc.vector.memset(ones32[:], 1.0)
            ps_w = psBig.tile([P, HID], f32, space="PSUM", tag="pcb")
            nc.tensor.matmul(out=ps_w[:], lhsT=ones32[:], rhs=nw_sb[:],
                             start=True, stop=True)
            w_rep = cpool.tile([P, HID], f32)
            nc.vector.tensor_copy(w_rep[:], ps_w[:])
            ps_b = psBig.tile([P, HID], f32, space="PSUM", tag="pcb")
            nc.tensor.matmul(out=ps_b[:], lhsT=ones32[:], rhs=nb_sb[:],
                             start=True, stop=True)
            b_rep = cpool.tile([P, HID], f32)
            nc.vector.tensor_copy(b_rep[:], ps_b[:])

        # ---- xdst table (SBUF-resident) -----------------------------------
        xdst_sb = cpool.tile([P, nw, HID], f16, tag="xdst")
        for gl in range(0, nw, 16):
            gln = min(16, nw - gl)
            not_t = bpool.tile([HID, 16 * P], f16, tag="not")
            nc.sync.dma_start(not_t[:, :gln * P],
                              node_ownT[:, gl * P:(gl + gln) * P])
            for gw in range(gl, gl + gln, 4):
                gn = min(4, gl + gln - gw)
                psX = psBig.tile([P, 4, HID], f32, space="PSUM", tag="pcb")
                for k in range(gn):
                    nc.tensor.matmul(out=psX[:, k, :],
                                     lhsT=not_t[:, (gw - gl + k) * P:
                                                (gw - gl + k + 1) * P],
                                     rhs=Wd[:],
                                     start=True, stop=True,
                                     skip_group_check=True)
                nc.scalar.activation(xdst_sb[:, gw:gw + gn, :], psX[:, :gn, :],
                                     AF.Prelu, alpha=1.0)

        # ---- xsrc gather table --------------------------------------------
        # 2048-node iterations: one big nodeT load (SP) + one big table write
        # (Act) each -- HWDGE charges ~0.6us per DMA instruction, so few big
        # DMAs beat many small ones. PSUM->SBUF copies alternate DVE/Act.
        starts = list(range(0, g.n_pad, 2048))  # last block may be partial
        nts = []
        for i, st in enumerate(starts):
            wdt = min(2048, g.n_pad - st)
            nt = bpool.tile([HID, 2048], f16, tag="nt", name=f"nt{i}",
                            bufs=3)
            nc.sync.dma_start(nt[:, :wdt], nodeT[:, st:st + wdt])
            nts.append(nt)
        for i, st in enumerate(starts):
            wdt = min(2048, g.n_pad - st)
            nq = wdt // 512
            nt = nts[i]
            xt = bpool.tile([P, 4, 4, HID], f16, tag="xt", bufs=3)
            for q in range(nq):
                ps = psBig.tile([P, 4, HID], f32, space="PSUM", tag="pcb")
                for j in range(4):
                    nc.tensor.matmul(out=ps[:, j, :],
                                     lhsT=nt[:, q * 512 + j * P:
                                             q * 512 + (j + 1) * P],
                                     rhs=Ws[:],
                                     start=True, stop=True,
                                     skip_group_check=True)
                if q % 2 == 0:
                    nc.vector.tensor_copy(xt[:, q, :, :], ps[:])
                else:
                    nc.scalar.activation(xt[:, q, :, :], ps[:],
                                         AF.Prelu, alpha=1.0)
            # dram row st + q*512 + p*4 + s  <-  xt[p, q, s, :]
            nc.scalar.dma_start(
                xsrc_tab[st:st + wdt, :]
                .rearrange("(q p s) h -> p q s h", p=P, s=4),
                xt[:, :nq, :, :])

        if debug:
            nc.sync.dma_start(dbg_xdst[:], xdst_sb[:])
            nc.sync.dma_start(dbg_resid[:], resid16[:])

        # ---- idx + dstr staging --------------------------------------------
        lo_sb = cpool.tile([P, lo_cols], mybir.dt.int16, tag="loidx")
        nc.sync.dma_start(lo_sb[:], lo_idx[:])
        hi_sb = cpool.tile([P, hi_cols], mybir.dt.int16, tag="hiidx")
        nc.sync.dma_start(hi_sb[:], hi_idx[:])
        dstr_sb = cpool.tile([P, total_chunks], f32, tag="dstr")
        nc.sync.dma_start(dstr_sb[:], dstr[:])

        off16 = {0: 0, 1: 0}
        qn = [0]

        def nextq():
            qn[0] = (qn[0] + 1) % 4
            return qn[0]

        # ---- edge slabs ----------------------------------------------------
        # Software-pipelined emission: each slab's input loads and one-hot
        # generation ("front") are emitted two slabs ahead of its compute +
        # flush ("back"), so the in-order per-engine sequencers never park a
        # next-slab load behind a previous slab's dependency waits.
        fronts = {}
        wins = {}

        def emit_front(s):
            sl = sched["slabs"][s]
            chunks = sl["chunks"]
            C = len(chunks)
            c0 = sl["chunk0"]

            xs_t = spool.tile([P, C, HID], f16, tag="xs", bufs=3,
                              name=f"xs{s}")
            for h in (0, 1):
                base = 0 if h == 0 else g.split
                idx_sb = lo_sb if h == 0 else hi_sb
                for (slot_off, n) in sl["calls"][h]:
                    if n == 0:
                        continue
                    nc.gpsimd.dma_gather(
                        out_ap=xs_t[:, slot_off:slot_off + n // P, :],
                        in_ap=xsrc_tab[base:g.n_pad, :],
                        idxs_ap=idx_sb[:, off16[h]:off16[h] + n // 16],
                        num_idxs=n, num_idxs_reg=n, elem_size=HID,
                        single_packet=(n <= 1024), queue_num=nextq(),
                    )
                    off16[h] += n // 16

            oet_t = spool.tile([NET, C * P], f16, tag="oet", bufs=3,
                               name=f"oet{s}")
            nc.sync.dma_start(oet_t[:], ohetT[:, c0 * P:(c0 + C) * P])

            oh_t = spool.tile([P, C, P], f16, tag="oh", bufs=3,
                              name=f"oh{s}")
            for ci in range(C):
                eng = nc.gpsimd if ci % 4 == 3 else nc.vector
                eng.tensor_scalar(
                    oh_t[:, ci, :], iota[:], dstr_sb[:, c0 + ci:c0 + ci + 1],
                    None, OP.is_equal)
            fronts[s] = (xs_t, oet_t, oh_t)

        def emit_back(s):
            sl = sched["slabs"][s]
            ws = sl["windows"]
            nwin = len(ws)
            chunks = sl["chunks"]
            C = len(chunks)
            c0 = sl["chunk0"]
            xs_t, oet_t, oh_t = fronts.pop(s)

            ohT_t = spool.tile([P, C, P], f16, tag="ohT", name=f"ohT{s}")
            rhs_t = spool.tile([P, C, 4 + HID], f16, tag="rhs",
                               name=f"rhs{s}")
            ex_ps = psEx.tile([P, C, H], f32, space="PSUM", tag="ex")

            ngrp = (C + 3) // 4
            for gi in range(ngrp):
                gn = min(4, C - 4 * gi)
                cs0 = 4 * gi
                # -- ohT for this group
                if True:
                    psT = psT4.tile([P, 4, P], f16, space="PSUM", tag="psT")
                    for k in range(gn):
                        nc.tensor.transpose(out=psT[:, k, :],
                                            in_=oh_t[:, cs0 + k, :],
                                            identity=ident[:])
                    if gi % 8 == 7:
                        nc.scalar.activation(ohT_t[:, cs0:cs0 + gn, :],
                                             psT[:, :gn, :], AF.Prelu,
                                             alpha=1.0)
                    else:
                        nc.vector.tensor_copy(ohT_t[:, cs0:cs0 + gn, :],
                                              psT[:, :gn, :])
                # -- combT accumulation in PSUM
                pcb = psBig.tile([HID, 4, P], f32, space="PSUM", tag="pcb")
                for k in range(gn):
                    ci = cs0 + k
                    wl = chunks[ci][0]
                    nc.tensor.matmul(out=pcb[:, k, :],
                                     lhsT=xdst_sb[:, ws[wl], :],
                                     rhs=ohT_t[:, ci, :],
                                     start=True, stop=False,
                                     skip_group_check=True)
                    nc.tensor.matmul(out=pcb[:, k, :], lhsT=emb_sb[:],
                                     rhs=oet_t[:, ci * P:(ci + 1) * P],
                                     start=False, stop=False,
                                     skip_group_check=True)
                    nc.tensor.matmul(out=pcb[:, k, :], lhsT=xs_t[:, ci, :],
                                     rhs=ident[:], start=False, stop=True,
                                     skip_group_check=True)
                combT = gpool.tile([HID, 4, P], f16, tag="comb")
                nc.scalar.activation(combT[:, :gn, :], pcb[:, :gn, :],
                                     AF.Prelu, alpha=0.2)
                if debug and s == 0:
                    nc.sync.dma_start(dbg_comb[:, cs0:cs0 + gn, :],
                                      combT[:, :gn, :])
                for k in range(gn):
                    ci = cs0 + k
                    nc.tensor.matmul(out=ex_ps[:, ci, :], lhsT=combT[:, k, :],
                                     rhs=att_sb[:], start=True, stop=True,
                                     skip_group_check=True)

            for g8 in range(0, C, 4):
                gn8 = min(4, C - g8)
                nc.scalar.activation(rhs_t[:, g8:g8 + gn8, 0:4],
                                     ex_ps[:, g8:g8 + gn8, :], AF.Exp)

            for ci in range(C):
                eng = (nc.vector if (ci % g.wmul_pool_mod
                                     == g.wmul_pool_mod - 1) else nc.gpsimd)
                eng.tensor_mul(
                    rhs_t[:, ci, 4:].rearrange("p (h d) -> p h d", h=H),
                    xs_t[:, ci, :].rearrange("p (h d) -> p h d", h=H),
                    rhs_t[:, ci, 0:4].unsqueeze(2).broadcast_to([P, H, HD]))

            # -- scatter into window accumulators
            win_t = psWin.tile([P, g.slab_w, 4 + HID], f32, space="PSUM",
                               tag="win", name=f"win{s}")
            n_per_win = [0] * nwin
            for (wl, h, slot) in chunks:
                n_per_win[wl] += 1
            # window-major scatter order: accumulation groups sharing a
            # PSUM bank must not interleave on hardware
            seen = [0] * nwin
            order = sorted(range(C), key=lambda ci: chunks[ci][0])
            for ci in order:
                wl = chunks[ci][0]
                first = seen[wl] == 0
                last = seen[wl] == n_per_win[wl] - 1
                seen[wl] += 1
                nc.tensor.matmul(out=win_t[:, wl, :], lhsT=oh_t[:, ci, :],
                                 rhs=rhs_t[:, ci, :], start=first, stop=last,
                                 skip_group_check=True)
            if debug and s == 0:
                nc.sync.dma_start(dbg_xs[:], xs_t[:])
                nc.sync.dma_start(dbg_ohT[:], ohT_t[:])
                nc.sync.dma_start(dbg_rhs[:], rhs_t[:])
                nc.sync.dma_start(dbg_oet[:], oet_t[:])
            wins[s] = (win_t, n_per_win)

        def emit_flush(s):
            # ---- flush windows (emitted one slab late so these dependency-
            # waiting ops never park in front of the next slab's work on the
            # in-order per-engine sequencers) --------------------------------
            sl = sched["slabs"][s]
            ws = sl["windows"]
            nwin = len(ws)
            win_t, n_per_win = wins.pop(s)
            if debug and s == 0:
                for _wl in range(len(ws)):
                    dbgw = fpool.tile([P, 4 + HID], f32, tag="yn")
                    nc.vector.tensor_copy(dbgw[:], win_t[:, _wl, :])
                    nc.sync.dma_start(dbg_win[:, _wl, :], dbgw[:])
            bn_sl = fpool.tile([P, nwin, 2], f32, tag="bnsl", name=f"bns{s}")
            y_l = []
            for wl, w in enumerate(ws):
                assert n_per_win[wl] > 0
                # f32: raw exp-sums can exceed the f16 max before the
                # normalization divide
                win16 = fpool.tile([P, 4 + HID], f32, tag="win16")
                nc.scalar.activation(win16[:], win_t[:, wl, :], AF.Prelu,
                                     alpha=1.0)
                sums = fpool.tile([P, 4], f32, tag="sums")
                nc.vector.tensor_scalar(sums[:], win16[:, 0:4], 1e-12, None,
                                        OP.max)
                rec = fpool.tile([P, 4], f32, tag="rec")
                nc.vector.reciprocal(rec[:], sums[:])
                if debug and s == 0:
                    nc.sync.dma_start(dbg_w16[wl], win16[:])
                aggn = fpool.tile([P, HID], f16, tag="aggn")
                nc.vector.tensor_mul(
                    aggn[:].rearrange("p (h d) -> p h d", h=H),
                    win16[:, 4:].rearrange("p (h d) -> p h d", h=H),
                    rec[:].unsqueeze(2).broadcast_to([P, H, HD]))
                if debug and s == 0:
                    nc.sync.dma_start(dbg_aggn[wl], aggn[:])
                psTf = psSm.tile([P, P], f16, space="PSUM", tag="pt")
                nc.tensor.transpose(out=psTf[:], in_=aggn[:], identity=ident[:])
                aggT = fpool.tile([HID, P], f16, tag="aggT")
                if wl % 2 == 0:
                    nc.scalar.activation(aggT[:], psTf[:], AF.Prelu, alpha=1.0)
                else:
                    nc.vector.tensor_copy(aggT[:], psTf[:])
                po = psSm.tile([P, HID], f32, space="PSUM", tag="pt")
                nc.tensor.matmul(out=po[:], lhsT=aggT[:], rhs=Wosc[:],
                                 start=True, stop=False,
                                 skip_group_check=True)
                nc.tensor.matmul(out=po[:], lhsT=ones_row[:], rhs=beta16[:],
                                 start=False, stop=True,
                                 skip_group_check=True)
                y = fpool.tile([P, HID], f32, tag="y", name=f"y{s}_{wl}",
                               bufs=g.slab_w + 1)
                nc.vector.tensor_add(y[:], po[:], resid16[:, w, :])
                y_l.append(y)
                if debug and s == 0:
                    nc.sync.dma_start(dbg_y[wl], y[:])
                bnst = fpool.tile([P, 6], f32, tag="bnst")
                nc.vector.bn_stats(bnst[:], y[:])
                nc.vector.bn_aggr(bn_sl[:, wl, :], bnst[:])
            sd = fpool.tile([P, nwin], f32, tag="sd", name=f"sd{s}")
            nc.scalar.activation(sd[:], bn_sl[:, :, 1], AF.Sqrt,
                                 bias=eps_col[:])
            rstd = fpool.tile([P, nwin], f32, tag="rstd", name=f"rs{s}")
            nc.vector.reciprocal(rstd[:], sd[:])
            if debug and s == 0:
                nc.sync.dma_start(dbg_bn[:], bn_sl[:])
                nc.sync.dma_start(dbg_sd[:, 0:nwin], sd[:])
                nc.sync.dma_start(dbg_sd[:, g.slab_w:g.slab_w + nwin],
                                  rstd[:])
            yn32 = fpool.tile([P, nwin, HID], f32, tag="yn", name=f"yn{s}")
            for wl, w in enumerate(ws):
                nc.vector.scalar_tensor_tensor(
                    yn32[:, wl, :], y_l[wl][:], bn_sl[:, wl, 0:1],
                    rstd[:, wl:wl + 1].broadcast_to([P, HID]),
                    OP.subtract, OP.mult)
                if not sched["skip_norm"]:
                    nc.vector.tensor_mul(yn32[:, wl, :], yn32[:, wl, :],
                                         w_rep[:])
                    nc.vector.tensor_add(yn32[:, wl, :], yn32[:, wl, :],
                                         b_rep[:])
            if debug and s == 0:
                nc.sync.dma_start(dbg_yn[:], yn32[:])
            nfull = sum(1 for w in ws if npc - w * P >= P)
            if nfull:
                nc.sync.dma_start(
                    out[ws[0] * P:ws[0] * P + nfull * P, :]
                    .rearrange("(w p) h -> p w h", p=P),
                    yn32[:, :nfull, :])
            for wl, w in enumerate(ws):
                rows = npc - w * P
                if rows < P:
                    nc.sync.dma_start(out[w * P:w * P + rows, :],
                                      yn32[:rows, wl, :])

        nslab = len(sched["slabs"])
        emit_front(0)
        if nslab > 1:
            emit_front(1)
        for s in range(nslab):
            if s + 2 < nslab:
                emit_front(s + 2)
            if s >= 1:
                emit_flush(s - 1)
            emit_back(s)
        emit_flush(nslab - 1)

    nc.compile()
    return nc


# ---------------------------------------------------------------------------
# Full-input entry point: shard, compile (cached), run SPMD on 8 cores,
# gather the output shards.
# ---------------------------------------------------------------------------
_CACHE = {}


def kernel(**inputs):
    N = int(np.asarray(inputs["node_embeddings"]).shape[0])
    n_cores = 8
    g = Geo(N=N, n_cores=n_cores)

    sched, in_maps = host_prep(g, **{k: np.asarray(v) for k, v in inputs.items()})

    key = (N, sched["total_chunks"], tuple(int(x) for x in sched["caps"].ravel()),
           sched["skip_norm"])
    if key not in _CACHE:
        _CACHE[key] = build_program(g, sched)
    nc = _CACHE[key]

    from concourse.bass_utils import run_bass_kernel_spmd
    res = run_bass_kernel_spmd(nc, in_maps, core_ids=list(range(n_cores)))
    out = np.concatenate([res.results[c]["out"] for c in range(n_cores)], axis=0)
    return out.astype(np.float32)
